# revision 11
# baseline (speedup 1.0000x reference)
"""Dilated (3x3, dilation=2) local-window attention for Trainium2.

Full inputs: x (32, 3136, 96) f32, W_qkv (288, 96) f32.
Sharding: data-parallel over batch, 4 images per core on 8 cores.

With dilation 2 the 56x56 image splits into 4 independent 28x28 parity
sub-lattices, each an ordinary 3x3 dilation-1 window attention
(zero-padded).  Host repacks x into padded parity layout
xt[97, par, 32, 32] (channel 96 = ones row driving the softmax
denominator; border rows/cols are zero pads).

Per parity sub-image (28 sub-rows x 28 cols), tokens processed in 14
blocks of 2 sub-rows (56 tokens), window = 4 padded rows x 32 = 128
partitions:
  - g = wqkL^T x (fused q^T k scores: S = x_pos^T g_tok), 2 matmuls.
  - pv: 8 matmuls transpose v windows to [4-row groups x 32, 97]
    (col 96 = den ones channel).
  - S: even blocks [128,56] natural row order; odd blocks ONE matmul
    with the lhsT free dim rotated (rows 2k+2,2k+3,2k,2k+1) so score
    partitions line up with the two pv groups the window straddles.
  - exp on ACT (scale folded), band mask on DVE/Pool (constant
    [128, 14, 56] tile, odd-block rotation baked into the mask).
  - AV: even blocks one matmul vs pv group k/2; odd blocks two matmuls
    (the straddled halves) accumulated into the same PSUM slot via
    start/stop groups - no drain-side add.
  - AV psum packed [97, 4 banks] per parity PAIR; one drain + one
    HBM DMA per pair (num/den division on host).

Emission interleaves next-image g/pv work and trailing AV blocks to
keep the PE queue fed while exp/mask latency drains.
"""

import numpy as np
import ml_dtypes

import concourse.bass as bass
import concourse.bacc as bacc
import concourse.tile as tile
from concourse import mybir
from concourse.bass_utils import run_bass_kernel_spmd

BF16 = mybir.dt.bfloat16
F32 = mybir.dt.float32

B = 32
NCORES = 8
BPC = B // NCORES   # images per core
H = 56
C = 96
N = H * H
SCALE = C ** -0.5
P = 4               # parity classes
R = 28              # sub-rows / cols per parity image
PR = 32             # padded width
RP = 32             # padded rows (row -1, 28 real, pad, 2 dummy)
NB = 14             # 2-sub-row blocks per parity
TOK = R * R         # 784 tokens per parity

_NC_CACHE = {}

_ACT, _DVE, _POOL = 0, 1, 2

# per-image drain/mask engine assignment (index = parity)
G_ENG = [_ACT, _DVE, _DVE, _DVE]
V_ENG = [_ACT, _DVE, _DVE, _ACT]
AV_ENG = [_ACT, _DVE]            # per h-pair
MASK_ENG = [_DVE, _DVE, _POOL, _POOL]


def _copy(nc, eng, dst, src):
    if eng == _ACT:
        nc.scalar.copy(dst, src)
    else:
        nc.vector.tensor_copy(dst, src)


def build_nc():
    nc = bacc.Bacc("TRN2", target_bir_lowering=False)
    xt_d = nc.dram_tensor("xt", [BPC, C + 1, P, RP, PR], BF16, kind="ExternalInput")
    wt_d = nc.dram_tensor("wt", [C + 1, 2 * C + 1], BF16, kind="ExternalInput")
    mk_d = nc.dram_tensor("mask", [128, NB, 56], BF16, kind="ExternalInput")
    o_d = nc.dram_tensor("o", [BPC, 2, C + 1, 4 * 392], BF16, kind="ExternalOutput")

    with tile.TileContext(nc) as tc:
        _body(tc, xt_d, wt_d, mk_d, o_d)
    nc.compile()
    return nc


def _body(tc, xt_d, wt_d, mk_d, o_d):
    nc = tc.nc
    with (
        tc.tile_pool(name="const", bufs=1) as const,
        tc.tile_pool(name="xpool", bufs=2) as xpool,
        tc.tile_pool(name="gpool", bufs=2) as gpool,
        tc.tile_pool(name="pvpool", bufs=5) as pvpool,
        tc.tile_pool(name="epool", bufs=2) as epool,
        tc.tile_pool(name="empool", bufs=3) as empool,
        tc.tile_pool(name="opool", bufs=2) as opool,
        tc.tile_pool(name="psa", bufs=2, space="PSUM") as psa,
        tc.tile_pool(name="pso", bufs=1, space="PSUM") as pso,
    ):
        w_sb = const.tile([C + 1, 2 * C + 1], BF16)
        nc.sync.dma_start(w_sb[:], wt_d[:])
        wqkL = w_sb[0:C, 0:C]              # lhsT for g = (wq^T wk)^T x
        wv_ext = w_sb[:, C:2 * C + 1]      # [97, 97] v + den-ones channel
        m_sb = const.tile([128, NB, 56], BF16)
        nc.sync.dma_start(m_sb[:], mk_d[:])

        xtp = [None, None]
        g_sb = [None, None]
        pv = {}
        em = {}
        ot = {}

        def xwin(xt, nparts, par, row0, nrows):
            # contiguous [nparts, nrows*32] window of xtp [97, 4, 32, 32]
            return bass.AP(tensor=xt.tensor,
                           offset=xt.offset + par * (RP * PR) + row0 * PR,
                           ap=[[list(xt.ap[0])[0], nparts], [1, nrows * PR]])

        def xwin_rot(xt, nparts, par, row0):
            # 4-row window enumerated rows row0+2, row0+3, row0, row0+1:
            # free dims [[-64, 2], [1, 64]] from offset at row0+2.
            return bass.AP(tensor=xt.tensor,
                           offset=xt.offset + par * (RP * PR) + (row0 + 2) * PR,
                           ap=[[list(xt.ap[0])[0], nparts],
                               [-2 * PR, 2], [1, 2 * PR]])

        def load_image(b):
            xtp[b % 2] = xpool.tile([C + 1, P, RP, PR], BF16, tag="xtp", name="xtp")
            nc.sync.dma_start(
                xtp[b % 2].rearrange("p a b c -> p (a b c)"),
                xt_d[b].rearrange("p a b c -> p (a b c)"))

        def g_parity(b, par):
            """g[:, par] = wqkL^T x (interior tokens only)."""
            xt = xtp[b % 2]
            if par == 0:
                g_sb[b % 2] = gpool.tile([C, P, R, R], BF16, tag="g", name="g")
            g = g_sb[b % 2]
            ps = psa.tile([128, 2, 512], F32, tag="a", name="a_ps")
            for h in range(2):
                nc.tensor.matmul(
                    ps[0:C, h, 0:392],
                    wqkL,
                    xt[0:C, par, 1 + 14 * h:15 + 14 * h, 1:29],
                    start=True, stop=True)
            gs = g[:, par]
            _copy(nc, G_ENG[par],
                  bass.AP(tensor=gs.tensor, offset=gs.offset,
                          ap=[list(g.ap[0]), [392, 2], [1, 392]]),
                  bass.AP(tensor=ps.tensor, offset=ps.offset,
                          ap=[[list(ps.ap[0])[0], C], [512, 2], [1, 392]]))

        def v_parity(b, par):
            """pv[(b,par)][128, 8, 97]: non-overlapping 4-row v groups."""
            xt = xtp[b % 2]
            ps = psa.tile([128, 2, 512], F32, tag="a", name="a_ps")
            for m in range(8):
                nc.tensor.matmul(
                    ps[:, m // 4, 97 * (m % 4):97 * (m % 4) + 97],
                    xwin(xt, C + 1, par, 4 * m, 4),
                    wv_ext,
                    start=True, stop=True)
            t = pvpool.tile([128, 8, C + 1], BF16, tag="pv", name="pvt")
            pv[(b, par)] = t
            _copy(nc, V_ENG[par],
                  bass.AP(tensor=t.tensor, offset=t.offset,
                          ap=[list(t.ap[0]), [388, 2], [1, 388]]),
                  bass.AP(tensor=ps.tensor, offset=ps.offset,
                          ap=[list(ps.ap[0]), [512, 2], [1, 388]]))

        def s_unit(b, par):
            """S -> exp -> mask for one parity."""
            xt = xtp[b % 2]
            g = g_sb[b % 2]
            ps = psa.tile([128, 2, 512], F32, tag="a", name="a_ps")
            for k in range(NB):
                cols = slice(56 * (k % 7), 56 * (k % 7) + 56)
                rhs = g[:, par, 2 * k:2 * k + 2, :]
                if k % 2 == 0:
                    nc.tensor.matmul(ps[:, k // 7, cols],
                                     xwin(xt, C, par, 2 * k, 4),
                                     rhs, start=True, stop=True)
                else:
                    # swapped halves so AV operand partitions align with pv
                    nc.tensor.matmul(ps[64:128, k // 7, cols],
                                     xwin(xt, C, par, 2 * k, 2),
                                     rhs, start=True, stop=True)
                    nc.tensor.matmul(ps[0:64, k // 7, cols],
                                     xwin(xt, C, par, 2 * k + 2, 2),
                                     rhs, start=True, stop=True)
            e_t = epool.tile([128, 2, 7, 56], BF16, tag="E", name="e_t")
            nc.scalar.activation(
                bass.AP(tensor=e_t.tensor, offset=e_t.offset,
                        ap=[list(e_t.ap[0]), [392, 2], [1, 392]]),
                bass.AP(tensor=ps.tensor, offset=ps.offset,
                        ap=[list(ps.ap[0]), [512, 2], [1, 392]]),
                mybir.ActivationFunctionType.Exp, scale=SCALE)
            t = empool.tile([128, 2, 7, 56], BF16, tag="EM", name="emt")
            em[(b, par)] = t
            if MASK_ENG[par] == _POOL:
                nc.gpsimd.tensor_mul(
                    t.rearrange("p a b c -> p (a b) c"),
                    e_t.rearrange("p a b c -> p (a b) c"),
                    m_sb[:])
            else:
                nc.vector.tensor_mul(
                    t.rearrange("p a b c -> p (a b) c"),
                    e_t.rearrange("p a b c -> p (a b) c"),
                    m_sb[:])

        def av_unit(b, par):
            """AV for one parity; drain+DMA per parity pair (q==1)."""
            q = par % 2
            h = par // 2
            if q == 0:
                ot[0] = pso.tile([C + 1, 4, 512], F32, tag="o", name="o_ps")
            t = ot[0]
            e = em.pop((b, par))
            v = pv.pop((b, par))
            for k in range(NB):
                dst = t[:, 2 * q + k // 7, 56 * (k % 7):56 * (k % 7) + 56]
                ek = e[:, k // 7, k % 7, :]
                j = k // 2
                if k % 2 == 0:
                    nc.tensor.matmul(dst, v[:, j, :], ek,
                                     start=True, stop=True)
                else:
                    # window straddles pv groups j and j+1: accumulate the
                    # two 64-partition halves into the same PSUM slot
                    nc.tensor.matmul(dst, v[64:128, j, :], ek[64:128],
                                     start=True, stop=False)
                    nc.tensor.matmul(dst, v[0:64, j + 1, :], ek[0:64],
                                     start=False, stop=True)
            if q == 1:
                osb = opool.tile([C + 1, 4, 392], BF16, tag="osb", name="osb")
                _copy(nc, AV_ENG[h],
                      bass.AP(tensor=osb.tensor, offset=osb.offset,
                              ap=[list(osb.ap[0]), [392, 4], [1, 392]]),
                      bass.AP(tensor=t.tensor, offset=t.offset,
                              ap=[list(t.ap[0]), [512, 4], [1, 392]]))
                nc.sync.dma_start(o_d[b, h],
                                  osb.rearrange("p a b -> p (a b)"))

        # ---- emission schedule ----
        load_image(0)
        for par in range(P):
            g_parity(0, par)
            v_parity(0, par)
        load_image(1)
        for b in range(BPC):
            s_unit(b, 0)
            s_unit(b, 1)
            av_unit(b, 0)
            s_unit(b, 2)
            av_unit(b, 1)
            s_unit(b, 3)
            if b + 1 < BPC:
                g_parity(b + 1, 0)
                v_parity(b + 1, 0)
                av_unit(b, 2)
                g_parity(b + 1, 1)
                v_parity(b + 1, 1)
                av_unit(b, 3)
                g_parity(b + 1, 2)
                v_parity(b + 1, 2)
                g_parity(b + 1, 3)
                v_parity(b + 1, 3)
                if b + 2 < BPC:
                    load_image(b + 2)
            else:
                av_unit(b, 2)
                av_unit(b, 3)


def _host_consts():
    # band mask [128, 56]: pos (k in 0..3, w in 0..31); token (j in 0..1,
    # wt in 0..27); valid iff k-j in {0,1,2} and w-wt in {0,1,2}
    k = np.arange(4)[:, None, None, None]
    w = np.arange(PR)[None, :, None, None]
    j = np.arange(2)[None, None, :, None]
    wt = np.arange(R)[None, None, None, :]
    m = ((k - j >= 0) & (k - j <= 2) & (w - wt >= 0) & (w - wt <= 2))
    m_even = m.astype(np.float32).reshape(4 * PR, 56)
    # odd blocks: physical partition row kk holds logical window row (kk+2)%4
    m_odd = m_even.reshape(4, PR, 56)[[2, 3, 0, 1]].reshape(4 * PR, 56)
    out = np.zeros((4 * PR, NB, 56), dtype=np.float32)
    for kb in range(NB):
        out[:, kb, :] = m_even if kb % 2 == 0 else m_odd
    return out.astype(ml_dtypes.bfloat16)


def _host_pack_x(x):
    """x (B, N, C) f32 -> (B, 97, 4, 32, 32) bf16 padded parity layout."""
    xr = x.reshape(B, H, H, C)
    out = np.zeros((B, C + 1, P, RP, PR), dtype=np.float32)
    for a in range(2):
        for c in range(2):
            par = 2 * a + c
            sub = xr[:, a::2, c::2, :]            # (B, 28, 28, C)
            out[:, 0:C, par, 1:29, 1:29] = sub.transpose(0, 3, 1, 2)
    out[:, C, :, :, :] = 1.0
    return out.astype(ml_dtypes.bfloat16)


def _host_pack_w(W_qkv):
    wq = W_qkv[0:C, :]
    wk = W_qkv[C:2 * C, :]
    wv = W_qkv[2 * C:3 * C, :]
    wt = np.zeros((C + 1, 2 * C + 1), dtype=np.float32)
    wt[0:C, 0:C] = wq.T @ wk                  # wqkL: g = wqkL^T x
    wt[0:C, C:2 * C] = wv.T                   # v = wv_ext^T x_ext
    wt[C, 2 * C] = 1.0                        # den ones channel
    return wt.astype(ml_dtypes.bfloat16)


def _host_unpack_o(o):
    """o (ncores, bpc, 2, 97, 1568) -> (ncores*bpc, N, C) f32 num/den."""
    o = np.asarray(o, dtype=np.float32)
    nc_, bpc = o.shape[0], o.shape[1]
    # [nc, b, h, 97, bank, 392] -> parity p = 2h + bank//2, 784 tokens
    o = o.reshape(nc_, bpc, 2, C + 1, 2, TOK)
    o = o.transpose(0, 1, 2, 4, 3, 5).reshape(nc_, bpc, P, C + 1, TOK)
    num = o[:, :, :, 0:C, :]
    den = o[:, :, :, C:C + 1, :]
    res = num / den                            # (nc, bpc, 4, 96, 784)
    res = res.reshape(nc_, bpc, 2, 2, C, R, R)
    y = np.zeros((nc_, bpc, H, H, C), dtype=np.float32)
    for a in range(2):
        for c in range(2):
            y[:, :, a::2, c::2, :] = res[:, :, a, c].transpose(0, 1, 3, 4, 2)
    return y.reshape(nc_ * bpc, N, C)


def kernel(x, W_qkv):
    x = np.asarray(x, dtype=np.float32)
    W_qkv = np.asarray(W_qkv, dtype=np.float32)

    if "nc" not in _NC_CACHE:
        _NC_CACHE["nc"] = build_nc()
    nc = _NC_CACHE["nc"]

    xt = _host_pack_x(x).reshape(NCORES, BPC, C + 1, P, RP, PR)
    wt = _host_pack_w(W_qkv)
    mk = _host_consts()

    in_maps = [{"xt": xt[i], "wt": wt, "mask": mk} for i in range(NCORES)]
    bkr = run_bass_kernel_spmd(nc, in_maps, list(range(NCORES)))
    _NC_CACHE["last_results"] = bkr
    o = np.stack([np.asarray(r["o"]) for r in bkr.results])
    return np.ascontiguousarray(_host_unpack_o(o).astype(np.float32))
